# revision 4
# baseline (speedup 1.0000x reference)
"""HardNegativeInfoNCE loss on 8 Trainium2 NeuronCores.

Strategy (full-io contract): host normalizes feats -> z, picks the random
positive per row (exact jax gumbel reproduction on CPU), and builds
one-hot label matrices. Rows are sharded 1024/core. Each core computes its
[1024, 8192] similarity block with full-speed fp32 (float32r) matmuls; the
same-label mask is applied INSIDE the matmul by augmenting the contraction
with +-10 * one-hot(label) (so masked entries come out as sim - 100).
Top-10 hard negatives per row via DVE max8 -> match_replace -> max8.
Per-core partial loss sums return to host, which reduces and divides.
"""

import numpy as np

TEMPERATURE = 0.07
HARD_K = 10
N, D, NUM_CLASSES = 8192, 256, 128
N_CORES = 8
ROWS_PER_CORE = N // N_CORES          # 1024
BLOCKS = ROWS_PER_CORE // 128         # 8
CHUNK = 512
N_CHUNKS = N // CHUNK                 # 16
MASK_S = 10.0                         # mask magnitude sqrt(100)

_COMPILED = {}


def _build_nc():
    import concourse.bacc as bacc
    import concourse.mybir as mybir
    from concourse import tile

    F32, BF16, F32R = mybir.dt.float32, mybir.dt.bfloat16, mybir.dt.float32r
    AF = mybir.ActivationFunctionType
    OP = mybir.AluOpType

    nc = bacc.Bacc()
    zt_d = nc.declare_dram_parameter("zt", [2, 128, N], BF16, isOutput=False)
    zrt_d = nc.declare_dram_parameter("zrt", [2, 128, ROWS_PER_CORE], BF16, isOutput=False)
    ohp_d = nc.declare_dram_parameter("ohp", [128, ROWS_PER_CORE], BF16, isOutput=False)
    ohn_d = nc.declare_dram_parameter("ohn", [128, N], BF16, isOutput=False)
    zr_d = nc.declare_dram_parameter("zr", [128, BLOCKS, D], F32, isOutput=False)
    zp_d = nc.declare_dram_parameter("zp", [128, BLOCKS, D], F32, isOutput=False)
    valid_d = nc.declare_dram_parameter("valid", [128, BLOCKS], F32, isOutput=False)
    out_d = nc.declare_dram_parameter("out", [128, BLOCKS], F32, isOutput=True)

    with tile.TileContext(nc) as tc:
        with (
            tc.tile_pool(name="big", bufs=1) as big,
            tc.tile_pool(name="spool", bufs=2) as spool,
            tc.tile_pool(name="small", bufs=4) as small,
            tc.tile_pool(name="acc", bufs=1) as accp,
            tc.tile_pool(name="psum", bufs=6, space="PSUM") as psp,
            tc.tile_pool(name="psw", bufs=1, space="PSUM") as psw,
        ):
            zt0 = big.tile([128, N], BF16, tag="zt0")
            zt1 = big.tile([128, N], BF16, tag="zt1")
            zrt0 = big.tile([128, ROWS_PER_CORE], BF16, tag="zrt0")
            zrt1 = big.tile([128, ROWS_PER_CORE], BF16, tag="zrt1")
            ohp = big.tile([128, ROWS_PER_CORE], BF16, tag="ohp")
            ohn = big.tile([128, N], BF16, tag="ohn")
            zr = big.tile([128, BLOCKS, D], F32, tag="zr")
            zp = big.tile([128, BLOCKS, D], F32, tag="zp")
            valid = big.tile([128, BLOCKS], F32, tag="valid")
            rall = accp.tile([128, BLOCKS], F32, tag="rall")

            nc.sync.dma_start(zt0[:], zt_d[0])
            nc.sync.dma_start(zt1[:], zt_d[1])
            nc.sync.dma_start(zrt0[:], zrt_d[0])
            nc.sync.dma_start(zrt1[:], zrt_d[1])
            nc.sync.dma_start(ohp[:], ohp_d[:])
            nc.sync.dma_start(ohn[:], ohn_d[:])
            nc.sync.dma_start(zr[:], zr_d[:])
            nc.sync.dma_start(zp[:], zp_d[:])
            nc.sync.dma_start(valid[:], valid_d[:])

            # Pre-consume DMA deps on PE so accumulation-group matmuls
            # carry no mid-group waits (HW LW sync-wait limit).
            warm = psw.tile([1, 8], F32, tag="warm")
            for t in (zt0, zt1, zrt0, zrt1, ohp, ohn):
                nc.tensor.matmul(warm[:1, :1], t[:, :1], t[:, :1], start=True, stop=True)

            inv_t = 1.0 / TEMPERATURE
            for b in range(BLOCKS):
                bs = b * 128
                S = spool.tile([128, N], F32, tag="S")
                for ch in range(N_CHUNKS):
                    cs = ch * CHUNK
                    ps = psp.tile([128, CHUNK], F32, tag="ps")
                    nc.tensor.matmul(ps[:], zrt0[:, bs:bs + 128], zt0[:, cs:cs + CHUNK],
                                     start=True, stop=False)
                    nc.tensor.matmul(ps[:], zrt1[:, bs:bs + 128], zt1[:, cs:cs + CHUNK],
                                     start=False, stop=False)
                    nc.tensor.matmul(ps[:], ohp[:, bs:bs + 128], ohn[:, cs:cs + CHUNK],
                                     start=False, stop=True)
                    nc.scalar.activation(S[:, cs:cs + CHUNK], ps[:], AF.Copy)

                m1 = small.tile([128, 8], F32, tag="m1")
                m2 = small.tile([128, 8], F32, tag="m2")
                nc.vector.max(out=m1[:], in_=S[:])
                nc.vector.match_replace(out=S[:], in_to_replace=m1[:], in_values=S[:],
                                        imm_value=-1000.0)
                nc.vector.max(out=m2[:], in_=S[:])

                e1 = small.tile([128, 8], F32, tag="e1")
                e2 = small.tile([128, 2], F32, tag="e2")
                d8 = small.tile([128, 1], F32, tag="d8")
                d2 = small.tile([128, 1], F32, tag="d2")
                nc.scalar.activation(e1[:], m1[:], AF.Exp, scale=inv_t, accum_out=d8[:])
                nc.scalar.activation(e2[:], m2[:, :2], AF.Exp, scale=inv_t, accum_out=d2[:])

                prod = small.tile([128, D], F32, tag="prod")
                pos = small.tile([128, 1], F32, tag="pos")
                nc.vector.tensor_mul(prod[:], zr[:, b, :], zp[:, b, :])
                nc.vector.reduce_sum(pos[:], prod[:], axis=mybir.AxisListType.X)
                num = small.tile([128, 1], F32, tag="num")
                nc.scalar.activation(num[:], pos[:], AF.Exp, scale=inv_t)

                den = small.tile([128, 1], F32, tag="den")
                nc.vector.tensor_add(den[:], num[:], d8[:])
                nc.vector.tensor_add(den[:], den[:], d2[:])
                nc.vector.tensor_scalar_max(den[:], den[:], 1e-8)
                rec = small.tile([128, 1], F32, tag="rec")
                nc.vector.reciprocal(rec[:], den[:])
                nc.vector.tensor_mul(rall[:, b:b + 1], num[:], rec[:])

            nc.vector.tensor_scalar_max(rall[:], rall[:], 1e-8)
            lnr = accp.tile([128, BLOCKS], F32, tag="lnr")
            nc.scalar.activation(lnr[:], rall[:], AF.Ln)
            masked = accp.tile([128, BLOCKS], F32, tag="masked")
            nc.vector.tensor_mul(masked[:], lnr[:], valid[:])
            nc.gpsimd.dma_start(out_d[:], masked[:])

    nc.finalize()
    return nc


def _host_prep(feats, labels):
    feats = np.asarray(feats, dtype=np.float32)
    labels = np.asarray(labels).astype(np.int64)
    n = feats.shape[0]

    norm = np.maximum(np.sqrt((feats.astype(np.float64) ** 2).sum(-1)), 1e-12)
    z = (feats / norm[:, None].astype(np.float32)).astype(np.float32)

    # exact reproduction of the reference's gumbel positive pick (CPU jax)
    import jax
    with jax.default_device(jax.devices("cpu")[0]):
        g = np.asarray(jax.random.gumbel(jax.random.key(42), (n, n), dtype=np.float32))
    same = labels[:, None] == labels[None, :]
    same_ns = same & ~np.eye(n, dtype=bool)
    gm = np.where(same_ns, g, -np.inf)
    pos_idx = gm.argmax(axis=1)
    del g, gm

    cnt = np.bincount(labels, minlength=NUM_CLASSES + 1)
    has_same = cnt[labels] >= 2
    has_diff = cnt[labels] < n
    valid = (has_same & has_diff).astype(np.float32)
    n_valid = max(int(valid.sum()), 1)

    zp_full = z[pos_idx]
    return z, zp_full, labels, valid, n_valid


def kernel(feats, labels):
    import ml_dtypes
    from concourse.bass_utils import run_bass_kernel_spmd

    z, zp_full, labels64, valid, n_valid = _host_prep(feats, labels)

    oh = np.zeros((NUM_CLASSES, N), np.float32)
    oh[labels64, np.arange(N)] = MASK_S
    ohp_full = oh.astype(ml_dtypes.bfloat16)
    ohn_full = (-oh).astype(ml_dtypes.bfloat16)
    zt = np.ascontiguousarray(z.T).reshape(2, 128, N).astype(ml_dtypes.bfloat16)

    in_maps = []
    for c in range(N_CORES):
        r0, r1 = c * ROWS_PER_CORE, (c + 1) * ROWS_PER_CORE
        zrows = z[r0:r1]
        in_maps.append({
            "zt": zt,
            "zrt": np.ascontiguousarray(zrows.T).reshape(2, 128, ROWS_PER_CORE).astype(ml_dtypes.bfloat16),
            "ohp": np.ascontiguousarray(ohp_full[:, r0:r1]),
            "ohn": ohn_full,
            "zr": np.ascontiguousarray(zrows.reshape(BLOCKS, 128, D).transpose(1, 0, 2)),
            "zp": np.ascontiguousarray(zp_full[r0:r1].reshape(BLOCKS, 128, D).transpose(1, 0, 2)),
            "valid": np.ascontiguousarray(valid[r0:r1].reshape(BLOCKS, 128).T),
        })

    if "nc" not in _COMPILED:
        _COMPILED["nc"] = _build_nc()
    res = run_bass_kernel_spmd(_COMPILED["nc"], in_maps, list(range(N_CORES)))

    total = np.float64(0.0)
    for c in range(N_CORES):
        total += np.float64(res.results[c]["out"].astype(np.float64).sum())
    loss = np.float32(-total / float(n_valid))
    return np.asarray(loss, dtype=np.float32)
